# revision 3
# baseline (speedup 1.0000x reference)
"""Bahdanau additive-attention kernel for Trainium2 (Bass/Tile), 8-core SPMD.

Reference computation (per batch b):
    att_en = en_seq[b] @ w_en            # [512, 256]
    att_de = de_seq[b] @ w_de            # [128, 256]
    mu[t, i] = sum_u nu[u] * tanh(att_de[t, u] + att_en[i, u])   # [128, 512]
    out[b] = softmax over i of (mu + (mask-1)*1e6)

Sharding: data-parallel over batch B=8 -> one batch per NeuronCore.

Per-core layout strategy:
  - u (UNITS=256) lives on partitions, split in 2 halves of 128.
  - att_enT[half] : [u=128, i=512]   att_deT[half] : [u=128, t=128]
  - For each t: DVE tensor_scalar_add broadcasts att_deT[:, t] over the
    free axis of att_enT -> S tile [128, 512], staged 8 t's at a time so
    one ScalarE Tanh instruction covers [128, 8192] (amortizes the ~300
    cycle ACT instruction overhead; ACT is the roofline engine here:
    8*128*512*256 tanh evals total).
  - nu-reduction over u on the TensorEngine: matmul with a sliding-window
    "diagonal" stationary operand W[:, 127-t:255-t] (zeros except column
    t = nu_half) accumulates row t of mu directly into one PSUM bank
    [t=128, i=512] across all 256 matmuls.
  - Softmax without max-subtraction (|mu| <= sum|nu| ~ 13, exp is safe in
    fp32) and with mask-as-multiply (exactly equivalent to the -1e6
    additive mask in fp32): P = exp(mu)*m / sum_i exp(mu)*m.
All math in fp32.
"""

import sys

if "/opt/trn_rl_repo" not in sys.path:
    sys.path.insert(0, "/opt/trn_rl_repo")

import numpy as np

import concourse.bass as bass
import concourse.mybir as mybir
import concourse.tile as tile
from concourse.masks import make_identity

B, T_EN, T_DE, D, U = 8, 512, 128, 512, 256
F32 = mybir.dt.float32
I32 = mybir.dt.int32
AF = mybir.ActivationFunctionType
ALU = mybir.AluOpType

# t's per staged tanh chunk (ACT instruction covers T_CHUNK*2*512 elements)
T_CHUNK = 8


def _split_sync_waits(nc, maxw=1):
    """walrus in this container rejects >1 sync wait per instruction
    ("Too many sync wait commands", CoreV3GenImpl setupSyncWait). Split
    excess waits onto preceding same-engine NoOps - semantically identical
    since engine streams execute in order."""
    ctr = 0
    for f in nc.m.functions:
        for bb in f.blocks:
            insts = list(bb.instructions)
            newlist = []
            changed = False
            for inst in insts:
                si = inst.sync_info
                waits = list(si.on_wait) if si is not None else []
                if len(waits) > maxw:
                    changed = True
                    chunks = [waits[i : i + maxw] for i in range(0, len(waits), maxw)]
                    for ch in chunks[:-1]:
                        ctr += 1
                        nop = mybir.InstNoOp(
                            name=f"waitsplit-{ctr}", ins=[], outs=[]
                        )
                        nop.engine = inst.engine
                        nop.sync_info = mybir.SyncInfo(on_wait=ch, on_update=[])
                        newlist.append(nop)
                    inst.sync_info = mybir.SyncInfo(
                        on_wait=chunks[-1], on_update=list(si.on_update)
                    )
                newlist.append(inst)
            if changed:
                bb.instructions = newlist
    return nc


def build_nc(split=True):
    nc = bass.Bass(trn_type="TRN2")
    en_d = nc.declare_dram_parameter("en", [T_EN, D], F32, isOutput=False)
    de_d = nc.declare_dram_parameter("de", [T_DE, D], F32, isOutput=False)
    wen_d = nc.declare_dram_parameter("w_en", [D, U], F32, isOutput=False)
    wde_d = nc.declare_dram_parameter("w_de", [D, U], F32, isOutput=False)
    nu_d = nc.declare_dram_parameter("nu", [U, 1], F32, isOutput=False)
    mask_d = nc.declare_dram_parameter("mask", [1, T_EN], I32, isOutput=False)
    out_d = nc.declare_dram_parameter("out", [T_DE, T_EN], F32, isOutput=True)

    with tile.TileContext(nc) as tc:
        with (
            tc.tile_pool(name="const", bufs=1) as constp,
            tc.tile_pool(name="io", bufs=1) as iop,
            tc.tile_pool(name="stage", bufs=2) as stagep,
            tc.tile_pool(name="ps", bufs=2, space="PSUM") as psp,
            tc.tile_pool(name="ps_mu", bufs=1, space="PSUM") as psmup,
        ):
            # ---------------- loads ----------------
            ident = constp.tile([128, 128], F32)
            make_identity(nc, ident[:])

            en_t = [iop.tile([128, D], F32, tag=f"en{i}", name=f"en_t{i}") for i in range(4)]
            for ic in range(4):
                nc.sync.dma_start(en_t[ic][:], en_d[ic * 128 : (ic + 1) * 128, :])
            de_t = iop.tile([128, D], F32, tag="de")
            nc.sync.dma_start(de_t[:], de_d[:, :])
            wen_t = [iop.tile([128, U], F32, tag=f"wen{i}", name=f"wen_t{i}") for i in range(4)]
            wde_t = [iop.tile([128, U], F32, tag=f"wde{i}", name=f"wde_t{i}") for i in range(4)]
            for dc in range(4):
                nc.sync.dma_start(wen_t[dc][:], wen_d[dc * 128 : (dc + 1) * 128, :])
                nc.sync.dma_start(wde_t[dc][:], wde_d[dc * 128 : (dc + 1) * 128, :])
            nu_t = [iop.tile([128, 1], F32, tag=f"nu{h}", name=f"nu_t{h}") for h in range(2)]
            for h in range(2):
                nc.sync.dma_start(nu_t[h][:], nu_d[h * 128 : (h + 1) * 128, :])
            mask_i = iop.tile([1, T_EN], I32, tag="mask_i")
            nc.sync.dma_start(mask_i[:], mask_d[:, :])

            # ---------------- transposes (PE) ----------------
            enT = [constp.tile([128, T_EN], F32, tag=f"enT{d}", name=f"enT{d}") for d in range(4)]
            for ic in range(4):
                for dc in range(4):
                    ps = psp.tile([128, 128], F32, tag="tr")
                    nc.tensor.transpose(
                        ps[:], en_t[ic][:, dc * 128 : (dc + 1) * 128], ident[:]
                    )
                    nc.vector.tensor_copy(
                        enT[dc][:, ic * 128 : (ic + 1) * 128], ps[:]
                    )
            deT = [constp.tile([128, T_DE], F32, tag=f"deT{d}", name=f"deT{d}") for d in range(4)]
            for dc in range(4):
                ps = psp.tile([128, 128], F32, tag="tr")
                nc.tensor.transpose(
                    ps[:], de_t[:, dc * 128 : (dc + 1) * 128], ident[:]
                )
                nc.vector.tensor_copy(deT[dc][:], ps[:])

            # ---------------- attention projections ----------------
            # att_enT[h][u, i] = sum_d w_en[d, u+128h] * en[i, d]
            att_enT = [constp.tile([128, T_EN], F32, tag=f"aenT{h}", name=f"att_enT{h}") for h in range(2)]
            for h in range(2):
                ps = psp.tile([128, T_EN], F32, tag="att")
                for dc in range(4):
                    nc.tensor.matmul(
                        ps[:],
                        wen_t[dc][:, h * 128 : (h + 1) * 128],
                        enT[dc][:],
                        start=(dc == 0),
                        stop=(dc == 3),
                    )
                nc.vector.tensor_copy(att_enT[h][:], ps[:])
            att_deT = [constp.tile([128, T_DE], F32, tag=f"adeT{h}", name=f"att_deT{h}") for h in range(2)]
            for h in range(2):
                ps = psp.tile([128, T_DE], F32, tag="att2")
                for dc in range(4):
                    nc.tensor.matmul(
                        ps[:],
                        wde_t[dc][:, h * 128 : (h + 1) * 128],
                        deT[dc][:],
                        start=(dc == 0),
                        stop=(dc == 3),
                    )
                nc.vector.tensor_copy(att_deT[h][:], ps[:])

            # ---------------- nu sliding window + mask broadcast ----------------
            # W[h] is zero except column 127 = nu_half; lhsT window
            # W[h][:, 127-t : 255-t] has nu in column t, zeros elsewhere.
            W = [constp.tile([128, 255], F32, tag=f"W{h}", name=f"W{h}") for h in range(2)]
            for h in range(2):
                nc.vector.memset(W[h][:], 0.0)
                nc.vector.tensor_copy(W[h][:, 127:128], nu_t[h][:])

            mask_f = constp.tile([1, T_EN], F32, tag="mask_f")
            nc.vector.tensor_copy(mask_f[:], mask_i[:])
            ones_r = constp.tile([1, 128], F32, tag="ones")
            nc.vector.memset(ones_r[:], 1.0)
            ps = psp.tile([128, T_EN], F32, tag="att")
            nc.tensor.matmul(ps[:], ones_r[:], mask_f[:], start=True, stop=True)
            maskb = constp.tile([128, T_EN], F32, tag="maskb")
            nc.vector.tensor_copy(maskb[:], ps[:])

            # ---------------- main loop ----------------
            mu_ps = psmup.tile([128, T_EN], F32)
            n_chunks = T_DE // T_CHUNK
            for c in range(n_chunks):
                S = stagep.tile([128, T_CHUNK * 2 * 512], F32, tag="S")
                T = stagep.tile([128, T_CHUNK * 2 * 512], F32, tag="T")
                for tl in range(T_CHUNK):
                    t = c * T_CHUNK + tl
                    for h in range(2):
                        off = (tl * 2 + h) * 512
                        nc.vector.tensor_scalar_add(
                            S[:, off : off + 512],
                            att_enT[h][:],
                            att_deT[h][:, t : t + 1],
                        )
                nc.scalar.activation(T[:], S[:], AF.Tanh)
                for tl in range(T_CHUNK):
                    t = c * T_CHUNK + tl
                    for h in range(2):
                        off = (tl * 2 + h) * 512
                        nc.tensor.matmul(
                            mu_ps[:],
                            W[h][:, 127 - t : 255 - t],
                            T[:, off : off + 512],
                            start=(t == 0 and h == 0),
                            stop=(t == T_DE - 1 and h == 1),
                        )

            # ---------------- softmax epilogue ----------------
            E = constp.tile([128, T_EN], F32, tag="E")
            nc.scalar.activation(E[:], mu_ps[:], AF.Exp)
            Em = constp.tile([128, T_EN], F32, tag="Em")
            nc.vector.tensor_mul(Em[:], E[:], maskb[:])
            ssum = constp.tile([128, 1], F32, tag="ssum")
            nc.vector.tensor_reduce(ssum[:], Em[:], axis=mybir.AxisListType.X, op=ALU.add)
            rinv = constp.tile([128, 1], F32, tag="rinv")
            nc.vector.reciprocal(rinv[:], ssum[:])
            P = constp.tile([128, T_EN], F32, tag="P")
            nc.vector.tensor_scalar_mul(P[:], Em[:], rinv[:])
            nc.sync.dma_start(out_d[:, :], P[:])

    if split:
        _split_sync_waits(nc)
    return nc


def _in_maps(en_seq, de_seq, mask_en, w_en, w_de, nu):
    en_seq = np.asarray(en_seq, dtype=np.float32)
    de_seq = np.asarray(de_seq, dtype=np.float32)
    mask_en = np.asarray(mask_en, dtype=np.int32)
    w_en = np.asarray(w_en, dtype=np.float32)
    w_de = np.asarray(w_de, dtype=np.float32)
    nu = np.asarray(nu, dtype=np.float32)
    return [
        {
            "en": en_seq[b],
            "de": de_seq[b],
            "mask": mask_en[b].reshape(1, T_EN),
            "w_en": w_en,
            "w_de": w_de,
            "nu": nu,
        }
        for b in range(B)
    ]


def kernel(en_seq, de_seq, mask_en, w_en, w_de, nu):
    from concourse.bass_utils import run_bass_kernel_spmd

    nc = build_nc()
    in_maps = _in_maps(en_seq, de_seq, mask_en, w_en, w_de, nu)
    res = run_bass_kernel_spmd(nc, in_maps, list(range(B)))
    out = np.stack([res.results[b]["out"] for b in range(B)], axis=0)
    return out.astype(np.float32)


# revision 4
# speedup vs baseline: 263.7192x; 263.7192x over previous
"""Bahdanau additive-attention kernel for Trainium2 (Bass/Tile), 8-core SPMD.

Reference computation (per batch b):
    att_en = en_seq[b] @ w_en            # [512, 256]
    att_de = de_seq[b] @ w_de            # [128, 256]
    mu[t, i] = sum_u nu[u] * tanh(att_de[t, u] + att_en[i, u])   # [128, 512]
    out[b] = softmax over i of (mu + (mask-1)*1e6)

Sharding: data-parallel over batch B=8 -> one batch per NeuronCore.

Per-core layout strategy:
  - u (UNITS=256) lives on partitions, split in 2 halves of 128.
  - att_enT[half] : [u=128, i=512]   att_deT[half] : [u=128, t=128]
  - For each t: DVE tensor_scalar_add broadcasts att_deT[:, t] over the
    free axis of att_enT -> S tile [128, 512], staged 8 t's at a time so
    one ScalarE Tanh instruction covers [128, 8192] (amortizes the ~300
    cycle ACT instruction overhead; ACT is the roofline engine here:
    8*128*512*256 tanh evals total).
  - nu-reduction over u on the TensorEngine: matmul with a sliding-window
    "diagonal" stationary operand W[:, 127-t:255-t] (zeros except column
    t = nu_half) accumulates row t of mu directly into one PSUM bank
    [t=128, i=512] across all 256 matmuls.
  - Softmax without max-subtraction (|mu| <= sum|nu| ~ 13, exp is safe in
    fp32) and with mask-as-multiply (exactly equivalent to the -1e6
    additive mask in fp32): P = exp(mu)*m / sum_i exp(mu)*m.
All math in fp32.
"""

import sys

if "/opt/trn_rl_repo" not in sys.path:
    sys.path.insert(0, "/opt/trn_rl_repo")

import numpy as np

import concourse.bass as bass
import concourse.mybir as mybir
import concourse.tile as tile
from concourse.masks import make_identity

B, T_EN, T_DE, D, U = 8, 512, 128, 512, 256
F32 = mybir.dt.float32
I32 = mybir.dt.int32
AF = mybir.ActivationFunctionType
ALU = mybir.AluOpType

# t's per staged tanh chunk (ACT instruction covers T_CHUNK*2*512 elements)
T_CHUNK = 8


def _split_sync_waits(nc, maxw=1):
    """walrus in this container rejects >1 sync wait per instruction
    ("Too many sync wait commands", CoreV3GenImpl setupSyncWait). Split
    excess waits onto preceding same-engine NoOps - semantically identical
    since engine streams execute in order."""
    ctr = 0
    for f in nc.m.functions:
        for bb in f.blocks:
            insts = list(bb.instructions)
            newlist = []
            changed = False
            for inst in insts:
                si = inst.sync_info
                waits = list(si.on_wait) if si is not None else []
                if len(waits) > maxw:
                    changed = True
                    chunks = [waits[i : i + maxw] for i in range(0, len(waits), maxw)]
                    for ch in chunks[:-1]:
                        ctr += 1
                        nop = mybir.InstNoOp(
                            name=f"waitsplit-{ctr}", ins=[], outs=[]
                        )
                        nop.engine = inst.engine
                        nop.sync_info = mybir.SyncInfo(on_wait=ch, on_update=[])
                        newlist.append(nop)
                    inst.sync_info = mybir.SyncInfo(
                        on_wait=chunks[-1], on_update=list(si.on_update)
                    )
                newlist.append(inst)
            if changed:
                bb.instructions = newlist
    return nc


def build_nc(split=True, reps=1):
    nc = bass.Bass(trn_type="TRN2")
    en_d = nc.declare_dram_parameter("en", [T_EN, D], F32, isOutput=False)
    de_d = nc.declare_dram_parameter("de", [T_DE, D], F32, isOutput=False)
    wen_d = nc.declare_dram_parameter("w_en", [D, U], F32, isOutput=False)
    wde_d = nc.declare_dram_parameter("w_de", [D, U], F32, isOutput=False)
    nu_d = nc.declare_dram_parameter("nu", [U, 1], F32, isOutput=False)
    mask_d = nc.declare_dram_parameter("mask", [1, T_EN], I32, isOutput=False)
    out_d = nc.declare_dram_parameter("out", [T_DE, T_EN], F32, isOutput=True)

    with tile.TileContext(nc) as tc:
        import contextlib
        loop_ctx = tc.For_i(0, reps, 1) if reps > 1 else contextlib.nullcontext()
        with (
            loop_ctx,
            tc.tile_pool(name="const", bufs=1) as constp,
            tc.tile_pool(name="io", bufs=1) as iop,
            tc.tile_pool(name="stage", bufs=2) as stagep,
            tc.tile_pool(name="ps", bufs=2, space="PSUM") as psp,
            tc.tile_pool(name="ps_mu", bufs=1, space="PSUM") as psmup,
        ):
            # ---------------- loads ----------------
            ident = constp.tile([128, 128], F32)
            make_identity(nc, ident[:])

            en_t = [iop.tile([128, D], F32, tag=f"en{i}", name=f"en_t{i}") for i in range(4)]
            for ic in range(4):
                nc.sync.dma_start(en_t[ic][:], en_d[ic * 128 : (ic + 1) * 128, :])
            de_t = iop.tile([128, D], F32, tag="de")
            nc.sync.dma_start(de_t[:], de_d[:, :])
            wen_t = [iop.tile([128, U], F32, tag=f"wen{i}", name=f"wen_t{i}") for i in range(4)]
            wde_t = [iop.tile([128, U], F32, tag=f"wde{i}", name=f"wde_t{i}") for i in range(4)]
            for dc in range(4):
                nc.sync.dma_start(wen_t[dc][:], wen_d[dc * 128 : (dc + 1) * 128, :])
                nc.sync.dma_start(wde_t[dc][:], wde_d[dc * 128 : (dc + 1) * 128, :])
            nu_t = [iop.tile([128, 1], F32, tag=f"nu{h}", name=f"nu_t{h}") for h in range(2)]
            for h in range(2):
                nc.sync.dma_start(nu_t[h][:], nu_d[h * 128 : (h + 1) * 128, :])
            mask_i = iop.tile([1, T_EN], I32, tag="mask_i")
            nc.sync.dma_start(mask_i[:], mask_d[:, :])

            # ---------------- transposes (PE) ----------------
            enT = [constp.tile([128, T_EN], F32, tag=f"enT{d}", name=f"enT{d}") for d in range(4)]
            for ic in range(4):
                for dc in range(4):
                    ps = psp.tile([128, 128], F32, tag="tr")
                    nc.tensor.transpose(
                        ps[:], en_t[ic][:, dc * 128 : (dc + 1) * 128], ident[:]
                    )
                    nc.vector.tensor_copy(
                        enT[dc][:, ic * 128 : (ic + 1) * 128], ps[:]
                    )
            deT = [constp.tile([128, T_DE], F32, tag=f"deT{d}", name=f"deT{d}") for d in range(4)]
            for dc in range(4):
                ps = psp.tile([128, 128], F32, tag="tr")
                nc.tensor.transpose(
                    ps[:], de_t[:, dc * 128 : (dc + 1) * 128], ident[:]
                )
                nc.vector.tensor_copy(deT[dc][:], ps[:])

            # ---------------- attention projections ----------------
            # att_enT[h][u, i] = sum_d w_en[d, u+128h] * en[i, d]
            att_enT = [constp.tile([128, T_EN], F32, tag=f"aenT{h}", name=f"att_enT{h}") for h in range(2)]
            for h in range(2):
                ps = psp.tile([128, T_EN], F32, tag="att")
                for dc in range(4):
                    nc.tensor.matmul(
                        ps[:],
                        wen_t[dc][:, h * 128 : (h + 1) * 128],
                        enT[dc][:],
                        start=(dc == 0),
                        stop=(dc == 3),
                    )
                nc.vector.tensor_copy(att_enT[h][:], ps[:])
            att_deT = [constp.tile([128, T_DE], F32, tag=f"adeT{h}", name=f"att_deT{h}") for h in range(2)]
            for h in range(2):
                ps = psp.tile([128, T_DE], F32, tag="att2")
                for dc in range(4):
                    nc.tensor.matmul(
                        ps[:],
                        wde_t[dc][:, h * 128 : (h + 1) * 128],
                        deT[dc][:],
                        start=(dc == 0),
                        stop=(dc == 3),
                    )
                nc.vector.tensor_copy(att_deT[h][:], ps[:])

            # ---------------- nu sliding window + mask broadcast ----------------
            # W[h] is zero except column 127 = nu_half; lhsT window
            # W[h][:, 127-t : 255-t] has nu in column t, zeros elsewhere.
            W = [constp.tile([128, 255], F32, tag=f"W{h}", name=f"W{h}") for h in range(2)]
            for h in range(2):
                nc.vector.memset(W[h][:], 0.0)
                nc.vector.tensor_copy(W[h][:, 127:128], nu_t[h][:])

            mask_f = constp.tile([1, T_EN], F32, tag="mask_f")
            nc.vector.tensor_copy(mask_f[:], mask_i[:])
            ones_r = constp.tile([1, 128], F32, tag="ones")
            nc.vector.memset(ones_r[:], 1.0)
            ps = psp.tile([128, T_EN], F32, tag="att")
            nc.tensor.matmul(ps[:], ones_r[:], mask_f[:], start=True, stop=True)
            maskb = constp.tile([128, T_EN], F32, tag="maskb")
            nc.vector.tensor_copy(maskb[:], ps[:])

            # ---------------- main loop ----------------
            mu_ps = psmup.tile([128, T_EN], F32)
            n_chunks = T_DE // T_CHUNK
            for c in range(n_chunks):
                S = stagep.tile([128, T_CHUNK * 2 * 512], F32, tag="S")
                T = stagep.tile([128, T_CHUNK * 2 * 512], F32, tag="T")
                for tl in range(T_CHUNK):
                    t = c * T_CHUNK + tl
                    for h in range(2):
                        off = (tl * 2 + h) * 512
                        nc.vector.tensor_scalar_add(
                            S[:, off : off + 512],
                            att_enT[h][:],
                            att_deT[h][:, t : t + 1],
                        )
                nc.scalar.activation(T[:], S[:], AF.Tanh)
                for tl in range(T_CHUNK):
                    t = c * T_CHUNK + tl
                    for h in range(2):
                        off = (tl * 2 + h) * 512
                        nc.tensor.matmul(
                            mu_ps[:],
                            W[h][:, 127 - t : 255 - t],
                            T[:, off : off + 512],
                            start=(t == 0 and h == 0),
                            stop=(t == T_DE - 1 and h == 1),
                        )

            # ---------------- softmax epilogue ----------------
            E = constp.tile([128, T_EN], F32, tag="E")
            nc.scalar.activation(E[:], mu_ps[:], AF.Exp)
            Em = constp.tile([128, T_EN], F32, tag="Em")
            nc.vector.tensor_mul(Em[:], E[:], maskb[:])
            ssum = constp.tile([128, 1], F32, tag="ssum")
            nc.vector.tensor_reduce(ssum[:], Em[:], axis=mybir.AxisListType.X, op=ALU.add)
            rinv = constp.tile([128, 1], F32, tag="rinv")
            nc.vector.reciprocal(rinv[:], ssum[:])
            P = constp.tile([128, T_EN], F32, tag="P")
            nc.vector.tensor_scalar_mul(P[:], Em[:], rinv[:])
            nc.sync.dma_start(out_d[:, :], P[:])

    if split:
        _split_sync_waits(nc)
    return nc


def _in_maps(en_seq, de_seq, mask_en, w_en, w_de, nu):
    en_seq = np.asarray(en_seq, dtype=np.float32)
    de_seq = np.asarray(de_seq, dtype=np.float32)
    mask_en = np.asarray(mask_en, dtype=np.int32)
    w_en = np.asarray(w_en, dtype=np.float32)
    w_de = np.asarray(w_de, dtype=np.float32)
    nu = np.asarray(nu, dtype=np.float32)
    return [
        {
            "en": en_seq[b],
            "de": de_seq[b],
            "mask": mask_en[b].reshape(1, T_EN),
            "w_en": w_en,
            "w_de": w_de,
            "nu": nu,
        }
        for b in range(B)
    ]


def kernel(en_seq, de_seq, mask_en, w_en, w_de, nu):
    from concourse.bass_utils import run_bass_kernel_spmd

    nc = build_nc()
    in_maps = _in_maps(en_seq, de_seq, mask_en, w_en, w_de, nu)
    res = run_bass_kernel_spmd(nc, in_maps, list(range(B)))
    out = np.stack([res.results[b]["out"] for b in range(B)], axis=0)
    return out.astype(np.float32)


# revision 5
# speedup vs baseline: 666.7975x; 2.5284x over previous
"""Bahdanau additive-attention kernel for Trainium2 (Bass/Tile), 8-core SPMD.

Reference computation (per batch b):
    att_en = en_seq[b] @ w_en            # [512, 256]
    att_de = de_seq[b] @ w_de            # [128, 256]
    mu[t, i] = sum_u nu[u] * tanh(att_de[t, u] + att_en[i, u])   # [128, 512]
    out[b] = softmax over i of (mu + (mask-1)*1e6)

Sharding: data-parallel over batch B=8 -> one batch per NeuronCore.

Per-core layout strategy:
  - u (UNITS=256) lives on partitions, split in 2 halves of 128.
  - att_enT[half] : [u=128, i=512]   att_deT[half] : [u=128, t=128]
  - For each t: DVE tensor_scalar_add broadcasts att_deT[:, t] over the
    free axis of att_enT -> S tile [128, 512], staged 8 t's at a time so
    one ScalarE Tanh instruction covers [128, 8192] (amortizes the ~300
    cycle ACT instruction overhead; ACT is the roofline engine here:
    8*128*512*256 tanh evals total).
  - nu-reduction over u on the TensorEngine: matmul with a sliding-window
    "diagonal" stationary operand W[:, 127-t:255-t] (zeros except column
    t = nu_half) accumulates row t of mu directly into one PSUM bank
    [t=128, i=512] across all 256 matmuls.
  - Softmax without max-subtraction (|mu| <= sum|nu| ~ 13, exp is safe in
    fp32) and with mask-as-multiply (exactly equivalent to the -1e6
    additive mask in fp32): P = exp(mu)*m / sum_i exp(mu)*m.
All math in fp32.
"""

import sys

if "/opt/trn_rl_repo" not in sys.path:
    sys.path.insert(0, "/opt/trn_rl_repo")

import numpy as np

import concourse.bass as bass
import concourse.mybir as mybir
import concourse.tile as tile
from concourse.masks import make_identity

B, T_EN, T_DE, D, U = 8, 512, 128, 512, 256
F32 = mybir.dt.float32
F16 = mybir.dt.float16
I32 = mybir.dt.int32
AF = mybir.ActivationFunctionType
ALU = mybir.AluOpType

# t's per staged tanh chunk (ACT instruction covers T_CHUNK*2*512 elements)
T_CHUNK = 8


def _split_sync_waits(nc, maxw=1):
    """walrus in this container rejects >1 sync wait per instruction
    ("Too many sync wait commands", CoreV3GenImpl setupSyncWait). Split
    excess waits onto preceding same-engine NoOps - semantically identical
    since engine streams execute in order."""
    ctr = 0
    for f in nc.m.functions:
        for bb in f.blocks:
            insts = list(bb.instructions)
            newlist = []
            changed = False
            for inst in insts:
                si = inst.sync_info
                waits = list(si.on_wait) if si is not None else []
                if len(waits) > maxw:
                    changed = True
                    chunks = [waits[i : i + maxw] for i in range(0, len(waits), maxw)]
                    for ch in chunks[:-1]:
                        ctr += 1
                        nop = mybir.InstNoOp(
                            name=f"waitsplit-{ctr}", ins=[], outs=[]
                        )
                        nop.engine = inst.engine
                        nop.sync_info = mybir.SyncInfo(on_wait=ch, on_update=[])
                        newlist.append(nop)
                    inst.sync_info = mybir.SyncInfo(
                        on_wait=chunks[-1], on_update=list(si.on_update)
                    )
                newlist.append(inst)
            if changed:
                bb.instructions = newlist
    return nc


def build_nc(split=True, reps=1):
    nc = bass.Bass(trn_type="TRN2")
    en_d = nc.declare_dram_parameter("en", [T_EN, D], F32, isOutput=False)
    de_d = nc.declare_dram_parameter("de", [T_DE, D], F32, isOutput=False)
    wen_d = nc.declare_dram_parameter("w_en", [D, U], F32, isOutput=False)
    wde_d = nc.declare_dram_parameter("w_de", [D, U], F32, isOutput=False)
    nu_d = nc.declare_dram_parameter("nu", [U, 1], F32, isOutput=False)
    mask_d = nc.declare_dram_parameter("mask", [1, T_EN], I32, isOutput=False)
    out_d = nc.declare_dram_parameter("out", [T_DE, T_EN], F32, isOutput=True)

    with tile.TileContext(nc) as tc:
        import contextlib
        loop_ctx = tc.For_i(0, reps, 1) if reps > 1 else contextlib.nullcontext()
        with (
            loop_ctx,
            tc.tile_pool(name="const", bufs=1) as constp,
            tc.tile_pool(name="io", bufs=1) as iop,
            tc.tile_pool(name="stage", bufs=2) as stagep,
            tc.tile_pool(name="ps", bufs=2, space="PSUM") as psp,
            tc.tile_pool(name="ps_mu", bufs=1, space="PSUM") as psmup,
        ):
            # ---------------- loads ----------------
            ident = constp.tile([128, 128], F32)
            make_identity(nc, ident[:])

            en_t = [iop.tile([128, D], F32, tag=f"en{i}", name=f"en_t{i}") for i in range(4)]
            for ic in range(4):
                nc.sync.dma_start(en_t[ic][:], en_d[ic * 128 : (ic + 1) * 128, :])
            de_t = iop.tile([128, D], F32, tag="de")
            nc.sync.dma_start(de_t[:], de_d[:, :])
            wen_t = [iop.tile([128, U], F32, tag=f"wen{i}", name=f"wen_t{i}") for i in range(4)]
            wde_t = [iop.tile([128, U], F32, tag=f"wde{i}", name=f"wde_t{i}") for i in range(4)]
            for dc in range(4):
                nc.sync.dma_start(wen_t[dc][:], wen_d[dc * 128 : (dc + 1) * 128, :])
                nc.sync.dma_start(wde_t[dc][:], wde_d[dc * 128 : (dc + 1) * 128, :])
            nu_t = [iop.tile([128, 1], F32, tag=f"nu{h}", name=f"nu_t{h}") for h in range(2)]
            for h in range(2):
                nc.sync.dma_start(nu_t[h][:], nu_d[h * 128 : (h + 1) * 128, :])
            mask_i = iop.tile([1, T_EN], I32, tag="mask_i")
            nc.sync.dma_start(mask_i[:], mask_d[:, :])

            # ---------------- transposes (PE) ----------------
            enT = [constp.tile([128, T_EN], F32, tag=f"enT{d}", name=f"enT{d}") for d in range(4)]
            for ic in range(4):
                for dc in range(4):
                    ps = psp.tile([128, 128], F32, tag="tr")
                    nc.tensor.transpose(
                        ps[:], en_t[ic][:, dc * 128 : (dc + 1) * 128], ident[:]
                    )
                    nc.vector.tensor_copy(
                        enT[dc][:, ic * 128 : (ic + 1) * 128], ps[:]
                    )
            deT = [constp.tile([128, T_DE], F32, tag=f"deT{d}", name=f"deT{d}") for d in range(4)]
            for dc in range(4):
                ps = psp.tile([128, 128], F32, tag="tr")
                nc.tensor.transpose(
                    ps[:], de_t[:, dc * 128 : (dc + 1) * 128], ident[:]
                )
                nc.vector.tensor_copy(deT[dc][:], ps[:])

            # ---------------- attention projections ----------------
            # att_enT[h][u, i] = sum_d w_en[d, u+128h] * en[i, d]
            att_enT = [constp.tile([128, T_EN], F32, tag=f"aenT{h}", name=f"att_enT{h}") for h in range(2)]
            for h in range(2):
                ps = psp.tile([128, T_EN], F32, tag="att")
                for dc in range(4):
                    nc.tensor.matmul(
                        ps[:],
                        wen_t[dc][:, h * 128 : (h + 1) * 128],
                        enT[dc][:],
                        start=(dc == 0),
                        stop=(dc == 3),
                    )
                nc.vector.tensor_copy(att_enT[h][:], ps[:])
            att_deT = [constp.tile([128, T_DE], F32, tag=f"adeT{h}", name=f"att_deT{h}") for h in range(2)]
            for h in range(2):
                ps = psp.tile([128, T_DE], F32, tag="att2")
                for dc in range(4):
                    nc.tensor.matmul(
                        ps[:],
                        wde_t[dc][:, h * 128 : (h + 1) * 128],
                        deT[dc][:],
                        start=(dc == 0),
                        stop=(dc == 3),
                    )
                nc.vector.tensor_copy(att_deT[h][:], ps[:])

            # ---------------- nu sliding window + mask broadcast ----------------
            # W[h] is zero except column 127 = nu_half; lhsT window
            # W[h][:, 127-t : 255-t] has nu in column t, zeros elsewhere.
            W = [constp.tile([128, 255], F16, tag=f"W{h}", name=f"W{h}") for h in range(2)]
            for h in range(2):
                nc.vector.memset(W[h][:], 0.0)
                nc.vector.tensor_copy(W[h][:, 127:128], nu_t[h][:])

            mask_f = constp.tile([1, T_EN], F32, tag="mask_f")
            nc.vector.tensor_copy(mask_f[:], mask_i[:])
            ones_r = constp.tile([1, 128], F32, tag="ones")
            nc.vector.memset(ones_r[:], 1.0)
            ps = psp.tile([128, T_EN], F32, tag="att")
            nc.tensor.matmul(ps[:], ones_r[:], mask_f[:], start=True, stop=True)
            maskb = constp.tile([128, T_EN], F32, tag="maskb")
            nc.vector.tensor_copy(maskb[:], ps[:])

            # ---------------- main loop ----------------
            mu_ps = psmup.tile([128, T_EN], F32)
            n_chunks = T_DE // T_CHUNK
            for c in range(n_chunks):
                S = stagep.tile([128, T_CHUNK * 2 * 512], F32, tag="S")
                T = stagep.tile([128, T_CHUNK * 2 * 512], F16, tag="T")
                for tl in range(T_CHUNK):
                    t = c * T_CHUNK + tl
                    for h in range(2):
                        off = (tl * 2 + h) * 512
                        nc.vector.tensor_scalar_add(
                            S[:, off : off + 512],
                            att_enT[h][:],
                            att_deT[h][:, t : t + 1],
                        )
                nc.scalar.activation(T[:], S[:], AF.Tanh)
                for tl in range(T_CHUNK):
                    t = c * T_CHUNK + tl
                    for h in range(2):
                        off = (tl * 2 + h) * 512
                        nc.tensor.matmul(
                            mu_ps[:],
                            W[h][:, 127 - t : 255 - t],
                            T[:, off : off + 512],
                            start=(t == 0 and h == 0),
                            stop=(t == T_DE - 1 and h == 1),
                        )

            # ---------------- softmax epilogue ----------------
            E = constp.tile([128, T_EN], F32, tag="E")
            nc.scalar.activation(E[:], mu_ps[:], AF.Exp)
            Em = constp.tile([128, T_EN], F32, tag="Em")
            nc.vector.tensor_mul(Em[:], E[:], maskb[:])
            ssum = constp.tile([128, 1], F32, tag="ssum")
            nc.vector.tensor_reduce(ssum[:], Em[:], axis=mybir.AxisListType.X, op=ALU.add)
            rinv = constp.tile([128, 1], F32, tag="rinv")
            nc.vector.reciprocal(rinv[:], ssum[:])
            P = constp.tile([128, T_EN], F32, tag="P")
            nc.vector.tensor_scalar_mul(P[:], Em[:], rinv[:])
            nc.sync.dma_start(out_d[:, :], P[:])

    if split:
        _split_sync_waits(nc)
    return nc


def _in_maps(en_seq, de_seq, mask_en, w_en, w_de, nu):
    en_seq = np.asarray(en_seq, dtype=np.float32)
    de_seq = np.asarray(de_seq, dtype=np.float32)
    mask_en = np.asarray(mask_en, dtype=np.int32)
    w_en = np.asarray(w_en, dtype=np.float32)
    w_de = np.asarray(w_de, dtype=np.float32)
    nu = np.asarray(nu, dtype=np.float32)
    return [
        {
            "en": en_seq[b],
            "de": de_seq[b],
            "mask": mask_en[b].reshape(1, T_EN),
            "w_en": w_en,
            "w_de": w_de,
            "nu": nu,
        }
        for b in range(B)
    ]


def kernel(en_seq, de_seq, mask_en, w_en, w_de, nu):
    from concourse.bass_utils import run_bass_kernel_spmd

    nc = build_nc()
    in_maps = _in_maps(en_seq, de_seq, mask_en, w_en, w_de, nu)
    res = run_bass_kernel_spmd(nc, in_maps, list(range(B)))
    out = np.stack([res.results[b]["out"] for b in range(B)], axis=0)
    return out.astype(np.float32)


# revision 7
# speedup vs baseline: 804.5870x; 1.2066x over previous
"""Bahdanau additive-attention kernel for Trainium2 (Bass/Tile), 8-core SPMD.

Reference computation (per batch b):
    att_en = en_seq[b] @ w_en            # [512, 256]
    att_de = de_seq[b] @ w_de            # [128, 256]
    mu[t, i] = sum_u nu[u] * tanh(att_de[t, u] + att_en[i, u])   # [128, 512]
    out[b] = softmax over i of (mu + (mask-1)*1e6)

Sharding: data-parallel over batch B=8 -> one batch per NeuronCore.

Per-core layout strategy:
  - u (UNITS=256) lives on partitions, split in 2 halves of 128.
  - att_enT[half] : [u=128, i=512]   att_deT[half] : [u=128, t=128]
  - For each t: DVE tensor_scalar_add broadcasts att_deT[:, t] over the
    free axis of att_enT -> S tile [128, 512], staged 8 t's at a time so
    one ScalarE Tanh instruction covers [128, 8192] (amortizes the ~300
    cycle ACT instruction overhead; ACT is the roofline engine here:
    8*128*512*256 tanh evals total).
  - nu-reduction over u on the TensorEngine: matmul with a sliding-window
    "diagonal" stationary operand W[:, 127-t:255-t] (zeros except column
    t = nu_half) accumulates row t of mu directly into one PSUM bank
    [t=128, i=512] across all 256 matmuls.
  - Softmax without max-subtraction (|mu| <= sum|nu| ~ 13, exp is safe in
    fp32) and with mask-as-multiply (exactly equivalent to the -1e6
    additive mask in fp32): P = exp(mu)*m / sum_i exp(mu)*m.
All math in fp32.
"""

import sys

if "/opt/trn_rl_repo" not in sys.path:
    sys.path.insert(0, "/opt/trn_rl_repo")

import numpy as np

import concourse.bass as bass
import concourse.mybir as mybir
import concourse.tile as tile
from concourse.masks import make_identity

B, T_EN, T_DE, D, U = 8, 512, 128, 512, 256
F32 = mybir.dt.float32
F16 = mybir.dt.float16
I32 = mybir.dt.int32
AF = mybir.ActivationFunctionType
ALU = mybir.AluOpType

# t's per staged tanh chunk (ACT instruction covers T_CHUNK*2*512 elements)
T_CHUNK = 16


def _split_sync_waits(nc, maxw=1):
    """walrus in this container rejects >1 sync wait per instruction
    ("Too many sync wait commands", CoreV3GenImpl setupSyncWait). Split
    excess waits onto preceding same-engine NoOps - semantically identical
    since engine streams execute in order."""
    ctr = 0
    for f in nc.m.functions:
        for bb in f.blocks:
            insts = list(bb.instructions)
            newlist = []
            changed = False
            for inst in insts:
                si = inst.sync_info
                waits = list(si.on_wait) if si is not None else []
                if len(waits) > maxw:
                    changed = True
                    chunks = [waits[i : i + maxw] for i in range(0, len(waits), maxw)]
                    for ch in chunks[:-1]:
                        ctr += 1
                        nop = mybir.InstNoOp(
                            name=f"waitsplit-{ctr}", ins=[], outs=[]
                        )
                        nop.engine = inst.engine
                        nop.sync_info = mybir.SyncInfo(on_wait=ch, on_update=[])
                        newlist.append(nop)
                    inst.sync_info = mybir.SyncInfo(
                        on_wait=chunks[-1], on_update=list(si.on_update)
                    )
                newlist.append(inst)
            if changed:
                bb.instructions = newlist
    return nc


def build_nc(split=True, reps=1):
    nc = bass.Bass(trn_type="TRN2")
    en_d = nc.declare_dram_parameter("en", [T_EN, D], F32, isOutput=False)
    de_d = nc.declare_dram_parameter("de", [T_DE, D], F32, isOutput=False)
    wen_d = nc.declare_dram_parameter("w_en", [D, U], F32, isOutput=False)
    wde_d = nc.declare_dram_parameter("w_de", [D, U], F32, isOutput=False)
    nu_d = nc.declare_dram_parameter("nu", [U, 1], F32, isOutput=False)
    mask_d = nc.declare_dram_parameter("mask", [1, T_EN], I32, isOutput=False)
    out_d = nc.declare_dram_parameter("out", [T_DE, T_EN], F32, isOutput=True)

    with tile.TileContext(nc) as tc:
        import contextlib
        loop_ctx = tc.For_i(0, reps, 1) if reps > 1 else contextlib.nullcontext()
        with (
            loop_ctx,
            tc.tile_pool(name="const", bufs=1) as constp,
            tc.tile_pool(name="io", bufs=1) as iop,
            tc.tile_pool(name="stage", bufs=2) as stagep,
            tc.tile_pool(name="ps", bufs=2, space="PSUM") as psp,
            tc.tile_pool(name="ps_mu", bufs=1, space="PSUM") as psmup,
        ):
            # ---------------- loads ----------------
            ident = constp.tile([128, 128], F32)
            make_identity(nc, ident[:])

            en_t = [iop.tile([128, D], F32, tag=f"en{i}", name=f"en_t{i}") for i in range(4)]
            for ic in range(4):
                nc.sync.dma_start(en_t[ic][:], en_d[ic * 128 : (ic + 1) * 128, :])
            de_t = iop.tile([128, D], F32, tag="de")
            nc.sync.dma_start(de_t[:], de_d[:, :])
            wen_t = [iop.tile([128, U], F32, tag=f"wen{i}", name=f"wen_t{i}") for i in range(4)]
            wde_t = [iop.tile([128, U], F32, tag=f"wde{i}", name=f"wde_t{i}") for i in range(4)]
            for dc in range(4):
                nc.sync.dma_start(wen_t[dc][:], wen_d[dc * 128 : (dc + 1) * 128, :])
                nc.sync.dma_start(wde_t[dc][:], wde_d[dc * 128 : (dc + 1) * 128, :])
            nu_t = [iop.tile([128, 1], F32, tag=f"nu{h}", name=f"nu_t{h}") for h in range(2)]
            for h in range(2):
                nc.sync.dma_start(nu_t[h][:], nu_d[h * 128 : (h + 1) * 128, :])
            mask_i = iop.tile([1, T_EN], I32, tag="mask_i")
            nc.sync.dma_start(mask_i[:], mask_d[:, :])

            # ---------------- transposes (PE) ----------------
            enT = [constp.tile([128, T_EN], F32, tag=f"enT{d}", name=f"enT{d}") for d in range(4)]
            for ic in range(4):
                for dc in range(4):
                    ps = psp.tile([128, 128], F32, tag="tr")
                    nc.tensor.transpose(
                        ps[:], en_t[ic][:, dc * 128 : (dc + 1) * 128], ident[:]
                    )
                    nc.vector.tensor_copy(
                        enT[dc][:, ic * 128 : (ic + 1) * 128], ps[:]
                    )
            deT = [constp.tile([128, T_DE], F32, tag=f"deT{d}", name=f"deT{d}") for d in range(4)]
            for dc in range(4):
                ps = psp.tile([128, 128], F32, tag="tr")
                nc.tensor.transpose(
                    ps[:], de_t[:, dc * 128 : (dc + 1) * 128], ident[:]
                )
                nc.vector.tensor_copy(deT[dc][:], ps[:])

            # ---------------- attention projections ----------------
            # att_enT[h][u, i] = sum_d w_en[d, u+128h] * en[i, d]
            att_enT = [constp.tile([128, T_EN], F16, tag=f"aenT{h}", name=f"att_enT{h}") for h in range(2)]
            for h in range(2):
                ps = psp.tile([128, T_EN], F32, tag="att")
                for dc in range(4):
                    nc.tensor.matmul(
                        ps[:],
                        wen_t[dc][:, h * 128 : (h + 1) * 128],
                        enT[dc][:],
                        start=(dc == 0),
                        stop=(dc == 3),
                    )
                nc.vector.tensor_copy(att_enT[h][:], ps[:])
            att_deT = [constp.tile([128, T_DE], F32, tag=f"adeT{h}", name=f"att_deT{h}") for h in range(2)]
            for h in range(2):
                ps = psp.tile([128, T_DE], F32, tag="att2")
                for dc in range(4):
                    nc.tensor.matmul(
                        ps[:],
                        wde_t[dc][:, h * 128 : (h + 1) * 128],
                        deT[dc][:],
                        start=(dc == 0),
                        stop=(dc == 3),
                    )
                nc.vector.tensor_copy(att_deT[h][:], ps[:])

            # ---------------- nu sliding window + mask broadcast ----------------
            # W[h] is zero except column 127 = nu_half; lhsT window
            # W[h][:, 127-t : 255-t] has nu in column t, zeros elsewhere.
            W = [constp.tile([128, 255], F16, tag=f"W{h}", name=f"W{h}") for h in range(2)]
            for h in range(2):
                nc.vector.memset(W[h][:], 0.0)
                nc.vector.tensor_copy(W[h][:, 127:128], nu_t[h][:])

            mask_f = constp.tile([1, T_EN], F32, tag="mask_f")
            nc.vector.tensor_copy(mask_f[:], mask_i[:])
            ones_r = constp.tile([1, 128], F32, tag="ones")
            nc.vector.memset(ones_r[:], 1.0)
            ps = psp.tile([128, T_EN], F32, tag="att")
            nc.tensor.matmul(ps[:], ones_r[:], mask_f[:], start=True, stop=True)
            maskb = constp.tile([128, T_EN], F32, tag="maskb")
            nc.vector.tensor_copy(maskb[:], ps[:])

            # ---------------- main loop ----------------
            mu_ps = psmup.tile([128, T_EN], F32)
            n_chunks = T_DE // T_CHUNK
            for c in range(n_chunks):
                S = stagep.tile([128, T_CHUNK * 2 * 512], F16, tag="S")
                T = stagep.tile([128, T_CHUNK * 2 * 512], F16, tag="T")
                for tl in range(T_CHUNK):
                    t = c * T_CHUNK + tl
                    for h in range(2):
                        off = (tl * 2 + h) * 512
                        nc.vector.tensor_scalar_add(
                            S[:, off : off + 512],
                            att_enT[h][:],
                            att_deT[h][:, t : t + 1],
                        )
                nc.scalar.activation(T[:], S[:], AF.Tanh)
                for tl in range(T_CHUNK):
                    t = c * T_CHUNK + tl
                    for h in range(2):
                        off = (tl * 2 + h) * 512
                        nc.tensor.matmul(
                            mu_ps[:],
                            W[h][:, 127 - t : 255 - t],
                            T[:, off : off + 512],
                            start=(t == 0 and h == 0),
                            stop=(t == T_DE - 1 and h == 1),
                        )

            # ---------------- softmax epilogue ----------------
            E = constp.tile([128, T_EN], F32, tag="E")
            nc.scalar.activation(E[:], mu_ps[:], AF.Exp)
            Em = constp.tile([128, T_EN], F32, tag="Em")
            nc.vector.tensor_mul(Em[:], E[:], maskb[:])
            ssum = constp.tile([128, 1], F32, tag="ssum")
            nc.vector.tensor_reduce(ssum[:], Em[:], axis=mybir.AxisListType.X, op=ALU.add)
            rinv = constp.tile([128, 1], F32, tag="rinv")
            nc.vector.reciprocal(rinv[:], ssum[:])
            P = constp.tile([128, T_EN], F32, tag="P")
            nc.vector.tensor_scalar_mul(P[:], Em[:], rinv[:])
            nc.sync.dma_start(out_d[:, :], P[:])

    if split:
        _split_sync_waits(nc)
    return nc


def _in_maps(en_seq, de_seq, mask_en, w_en, w_de, nu):
    en_seq = np.asarray(en_seq, dtype=np.float32)
    de_seq = np.asarray(de_seq, dtype=np.float32)
    mask_en = np.asarray(mask_en, dtype=np.int32)
    w_en = np.asarray(w_en, dtype=np.float32)
    w_de = np.asarray(w_de, dtype=np.float32)
    nu = np.asarray(nu, dtype=np.float32)
    return [
        {
            "en": en_seq[b],
            "de": de_seq[b],
            "mask": mask_en[b].reshape(1, T_EN),
            "w_en": w_en,
            "w_de": w_de,
            "nu": nu,
        }
        for b in range(B)
    ]


def kernel(en_seq, de_seq, mask_en, w_en, w_de, nu):
    from concourse.bass_utils import run_bass_kernel_spmd

    nc = build_nc()
    in_maps = _in_maps(en_seq, de_seq, mask_en, w_en, w_de, nu)
    res = run_bass_kernel_spmd(nc, in_maps, list(range(B)))
    out = np.stack([res.results[b]["out"] for b in range(B)], axis=0)
    return out.astype(np.float32)


# revision 9
# speedup vs baseline: 990.9454x; 1.2316x over previous
"""Bahdanau additive-attention kernel for Trainium2 (Bass/Tile), 8-core SPMD.

Reference computation (per batch b):
    att_en = en_seq[b] @ w_en            # [512, 256]
    att_de = de_seq[b] @ w_de            # [128, 256]
    mu[t, i] = sum_u nu[u] * tanh(att_de[t, u] + att_en[i, u])   # [128, 512]
    out[b] = softmax over i of (mu + (mask-1)*1e6)

Sharding: data-parallel over batch B=8 -> one batch per NeuronCore.

Mask compaction (host side): encoder positions with mask==0 contribute
exactly 0 to the softmax output (exp(x-1e6) == 0.0 in fp32), so the host
gathers only the unmasked en rows, padded to L_pad = max over batches
(rounded up to a multiple of 8). The kernel computes over L_pad columns;
padding columns carry mask=0 and are zeroed by the same mask-multiply that
implements the reference's additive -1e6 mask. The host scatters results
back to the full [128, 512] grid (masked positions = 0, exactly matching
the reference). This cuts tanh work - the ScalarE roofline of this
problem - roughly in half for p=0.5 masks.

Per-core layout:
  - u (UNITS=256) on partitions, 2 halves of 128.
  - att_enT[half]: [u=128, i=L]  fp16;  att_deT[half]: [u=128, t=128] fp32.
  - For each t: DVE tensor_scalar_add (4x mode, fp16) broadcasts
    att_deT[:, t] over att_enT -> S slab; T_CHUNK t's staged so one
    ScalarE Tanh covers [128, T_CHUNK*2*L] (ACT instr overhead amortized).
  - nu-reduction over u on TensorE: sliding-window "diagonal" stationary
    W[:, 127-t:255-t] (zeros except column t = nu_half, fp16) accumulates
    row t of mu into one PSUM bank [t=128, i=L] across all 256 matmuls.
  - Softmax without max-subtraction (|mu| <= sum|nu| ~ 13; fp32 exp safe),
    mask as multiply, DVE reciprocal.
"""

import sys

if "/opt/trn_rl_repo" not in sys.path:
    sys.path.insert(0, "/opt/trn_rl_repo")

import numpy as np

import concourse.bass as bass
import concourse.mybir as mybir
import concourse.tile as tile
from concourse.masks import make_identity

B, T_EN, T_DE, D, U = 8, 512, 128, 512, 256
F32 = mybir.dt.float32
F16 = mybir.dt.float16
I32 = mybir.dt.int32
AF = mybir.ActivationFunctionType
ALU = mybir.AluOpType

T_CHUNK = 16  # t's per staged tanh chunk


def _split_sync_waits(nc, maxw=1):
    """walrus in this container rejects >1 sync wait per instruction
    ("Too many sync wait commands", CoreV3GenImpl setupSyncWait). Split
    excess waits onto preceding same-engine NoOps - semantically identical
    since engine streams execute in order."""
    ctr = 0
    for f in nc.m.functions:
        for bb in f.blocks:
            insts = list(bb.instructions)
            newlist = []
            changed = False
            for inst in insts:
                si = inst.sync_info
                waits = list(si.on_wait) if si is not None else []
                if len(waits) > maxw:
                    changed = True
                    chunks = [waits[i : i + maxw] for i in range(0, len(waits), maxw)]
                    for ch in chunks[:-1]:
                        ctr += 1
                        nop = mybir.InstNoOp(name=f"waitsplit-{ctr}", ins=[], outs=[])
                        nop.engine = inst.engine
                        nop.sync_info = mybir.SyncInfo(on_wait=ch, on_update=[])
                        newlist.append(nop)
                    inst.sync_info = mybir.SyncInfo(
                        on_wait=chunks[-1], on_update=list(si.on_update)
                    )
                newlist.append(inst)
            if changed:
                bb.instructions = newlist
    return nc


def build_nc(split=True, reps=1, l_pad=T_EN):
    L = l_pad
    assert L % 8 == 0 and 8 <= L <= T_EN
    n_iblk = (L + 127) // 128  # en row blocks (last may be ragged)

    nc = bass.Bass(trn_type="TRN2")
    en_d = nc.declare_dram_parameter("en", [L, D], F32, isOutput=False)
    de_d = nc.declare_dram_parameter("de", [T_DE, D], F32, isOutput=False)
    wen_d = nc.declare_dram_parameter("w_en", [D, U], F32, isOutput=False)
    wde_d = nc.declare_dram_parameter("w_de", [D, U], F32, isOutput=False)
    nu_d = nc.declare_dram_parameter("nu", [U, 1], F32, isOutput=False)
    mask_d = nc.declare_dram_parameter("mask", [1, L], I32, isOutput=False)
    out_d = nc.declare_dram_parameter("out", [T_DE, L], F32, isOutput=True)

    with tile.TileContext(nc) as tc:
        import contextlib

        loop_ctx = tc.For_i(0, reps, 1) if reps > 1 else contextlib.nullcontext()
        with (
            loop_ctx,
            tc.tile_pool(name="const", bufs=1) as constp,
            tc.tile_pool(name="io", bufs=1) as iop,
            tc.tile_pool(name="stage", bufs=2) as stagep,
            tc.tile_pool(name="ps", bufs=2, space="PSUM") as psp,
            tc.tile_pool(name="ps_mu", bufs=1, space="PSUM") as psmup,
        ):
            # ---------------- loads ----------------
            ident = constp.tile([128, 128], F32)
            make_identity(nc, ident[:])

            en_t = [
                iop.tile([128, D], F32, tag=f"en{i}", name=f"en_t{i}")
                for i in range(n_iblk)
            ]
            for ib in range(n_iblk):
                rem = min(128, L - ib * 128)
                nc.sync.dma_start(
                    en_t[ib][:rem, :], en_d[ib * 128 : ib * 128 + rem, :]
                )
            de_t = iop.tile([128, D], F32, tag="de")
            nc.sync.dma_start(de_t[:], de_d[:, :])
            wen_t = [
                iop.tile([128, U], F32, tag=f"wen{i}", name=f"wen_t{i}")
                for i in range(4)
            ]
            wde_t = [
                iop.tile([128, U], F32, tag=f"wde{i}", name=f"wde_t{i}")
                for i in range(4)
            ]
            for dc in range(4):
                nc.sync.dma_start(wen_t[dc][:], wen_d[dc * 128 : (dc + 1) * 128, :])
                nc.sync.dma_start(wde_t[dc][:], wde_d[dc * 128 : (dc + 1) * 128, :])
            nu_t = [
                iop.tile([128, 1], F32, tag=f"nu{h}", name=f"nu_t{h}") for h in range(2)
            ]
            for h in range(2):
                nc.sync.dma_start(nu_t[h][:], nu_d[h * 128 : (h + 1) * 128, :])
            mask_i = iop.tile([1, L], I32, tag="mask_i")
            nc.sync.dma_start(mask_i[:], mask_d[:, :])

            # ---------------- transposes (PE) ----------------
            enT = [
                constp.tile([128, L], F32, tag=f"enT{d}", name=f"enT{d}")
                for d in range(4)
            ]
            for ib in range(n_iblk):
                rem = min(128, L - ib * 128)
                for dc in range(4):
                    ps = psp.tile([128, 128], F32, tag="tr")
                    nc.tensor.transpose(
                        ps[:, :rem],
                        en_t[ib][:rem, dc * 128 : (dc + 1) * 128],
                        ident[:rem, :rem],
                    )
                    nc.vector.tensor_copy(
                        enT[dc][:, ib * 128 : ib * 128 + rem], ps[:, :rem]
                    )
            deT = [
                constp.tile([128, T_DE], F32, tag=f"deT{d}", name=f"deT{d}")
                for d in range(4)
            ]
            for dc in range(4):
                ps = psp.tile([128, 128], F32, tag="tr")
                nc.tensor.transpose(ps[:], de_t[:, dc * 128 : (dc + 1) * 128], ident[:])
                nc.vector.tensor_copy(deT[dc][:], ps[:])

            # ---------------- attention projections ----------------
            att_enT = [
                constp.tile([128, L], F16, tag=f"aenT{h}", name=f"att_enT{h}")
                for h in range(2)
            ]
            for h in range(2):
                ps = psp.tile([128, L], F32, tag="att")
                for dc in range(4):
                    nc.tensor.matmul(
                        ps[:],
                        wen_t[dc][:, h * 128 : (h + 1) * 128],
                        enT[dc][:],
                        start=(dc == 0),
                        stop=(dc == 3),
                    )
                nc.vector.tensor_copy(att_enT[h][:], ps[:])
            att_deT = [
                constp.tile([128, T_DE], F32, tag=f"adeT{h}", name=f"att_deT{h}")
                for h in range(2)
            ]
            for h in range(2):
                ps = psp.tile([128, T_DE], F32, tag="att2")
                for dc in range(4):
                    nc.tensor.matmul(
                        ps[:],
                        wde_t[dc][:, h * 128 : (h + 1) * 128],
                        deT[dc][:],
                        start=(dc == 0),
                        stop=(dc == 3),
                    )
                nc.vector.tensor_copy(att_deT[h][:], ps[:])

            # ---------------- nu sliding window + mask broadcast ----------------
            W = [
                constp.tile([128, 255], F16, tag=f"W{h}", name=f"W{h}")
                for h in range(2)
            ]
            for h in range(2):
                nc.vector.memset(W[h][:], 0.0)
                nc.vector.tensor_copy(W[h][:, 127:128], nu_t[h][:])

            mask_f = constp.tile([1, L], F32, tag="mask_f")
            nc.vector.tensor_copy(mask_f[:], mask_i[:])
            ones_r = constp.tile([1, 128], F32, tag="ones")
            nc.vector.memset(ones_r[:], 1.0)
            ps = psp.tile([128, L], F32, tag="att")
            nc.tensor.matmul(ps[:], ones_r[:], mask_f[:], start=True, stop=True)
            maskb = constp.tile([128, L], F32, tag="maskb")
            nc.vector.tensor_copy(maskb[:], ps[:])

            # ---------------- main loop ----------------
            mu_ps = psmup.tile([128, L], F32)
            n_chunks = T_DE // T_CHUNK
            for c in range(n_chunks):
                S = stagep.tile([128, T_CHUNK * 2 * L], F16, tag="S")
                T = stagep.tile([128, T_CHUNK * 2 * L], F16, tag="T")
                for tl in range(T_CHUNK):
                    t = c * T_CHUNK + tl
                    for h in range(2):
                        off = (tl * 2 + h) * L
                        nc.vector.tensor_scalar_add(
                            S[:, off : off + L],
                            att_enT[h][:],
                            att_deT[h][:, t : t + 1],
                        )
                nc.scalar.activation(T[:], S[:], AF.Tanh)
                for tl in range(T_CHUNK):
                    t = c * T_CHUNK + tl
                    for h in range(2):
                        off = (tl * 2 + h) * L
                        nc.tensor.matmul(
                            mu_ps[:],
                            W[h][:, 127 - t : 255 - t],
                            T[:, off : off + L],
                            start=(t == 0 and h == 0),
                            stop=(t == T_DE - 1 and h == 1),
                        )

            # ---------------- softmax epilogue ----------------
            E = constp.tile([128, L], F32, tag="E")
            nc.scalar.activation(E[:], mu_ps[:], AF.Exp)
            Em = constp.tile([128, L], F32, tag="Em")
            nc.vector.tensor_mul(Em[:], E[:], maskb[:])
            ssum = constp.tile([128, 1], F32, tag="ssum")
            nc.vector.tensor_reduce(
                ssum[:], Em[:], axis=mybir.AxisListType.X, op=ALU.add
            )
            rinv = constp.tile([128, 1], F32, tag="rinv")
            nc.vector.reciprocal(rinv[:], ssum[:])
            P = constp.tile([128, L], F32, tag="P")
            nc.vector.tensor_scalar_mul(P[:], Em[:], rinv[:])
            nc.sync.dma_start(out_d[:, :], P[:])

    if split:
        _split_sync_waits(nc)
    return nc


def _compact(en_seq, mask_en):
    """Host-side gather of unmasked en rows, padded to a shared L_pad."""
    idxs = [np.nonzero(mask_en[b])[0] for b in range(B)]
    lmax = max((len(ix) for ix in idxs), default=0)
    l_pad = max(8, -(-lmax // 8) * 8)
    en_c = np.zeros((B, l_pad, D), dtype=np.float32)
    mask_c = np.zeros((B, 1, l_pad), dtype=np.int32)
    for b in range(B):
        n = len(idxs[b])
        en_c[b, :n] = en_seq[b, idxs[b]]
        mask_c[b, 0, :n] = 1
    return idxs, l_pad, en_c, mask_c


def _in_maps(en_c, de_seq, mask_c, w_en, w_de, nu):
    return [
        {
            "en": en_c[b],
            "de": de_seq[b],
            "mask": mask_c[b],
            "w_en": w_en,
            "w_de": w_de,
            "nu": nu,
        }
        for b in range(B)
    ]


def kernel(en_seq, de_seq, mask_en, w_en, w_de, nu):
    from concourse.bass_utils import run_bass_kernel_spmd

    en_seq = np.asarray(en_seq, dtype=np.float32)
    de_seq = np.asarray(de_seq, dtype=np.float32)
    mask_en = np.asarray(mask_en, dtype=np.int32)
    w_en = np.asarray(w_en, dtype=np.float32)
    w_de = np.asarray(w_de, dtype=np.float32)
    nu = np.asarray(nu, dtype=np.float32)

    idxs, l_pad, en_c, mask_c = _compact(en_seq, mask_en)

    out = np.zeros((B, T_DE, T_EN), dtype=np.float32)
    empty = [b for b in range(B) if len(idxs[b]) == 0]
    if empty:
        # Degenerate: a fully-masked batch. The reference's logits are
        # mu - 1e6; softmax over the fp32-rounded values. Compute on host.
        for b in empty:
            att_en = en_seq[b] @ w_en
            att_de = de_seq[b] @ w_de
            mu = np.tanh(att_de[:, None, :] + att_en[None, :, :]) @ nu[:, 0]
            lg = (mu - 1e6).astype(np.float32)
            lg = lg - lg.max(axis=-1, keepdims=True)
            e = np.exp(lg)
            out[b] = e / e.sum(axis=-1, keepdims=True)

    nc = build_nc(l_pad=l_pad)
    in_maps = _in_maps(en_c, de_seq, mask_c, w_en, w_de, nu)
    res = run_bass_kernel_spmd(nc, in_maps, list(range(B)))
    for b in range(B):
        if b in empty:
            continue
        n = len(idxs[b])
        out[b][:, idxs[b]] = res.results[b]["out"][:, :n]
    return out
